# revision 45
# baseline (speedup 1.0000x reference)
"""Trainium2 Bass kernel for a dense transformer block (B=4, T=2048, C=1024, H=16).

Sharding: zero-collective. Each of the 8 cores owns (batch b, query parity par):
  core c -> b = c//2, par = c%2; query chunks = 256-token chunks (2j+par), j=0..3.
Parity striping balances causal attention work exactly: every core runs chunks
whose k-tile counts are (4, 8, 12, 16) -- a uniform SPMD instruction stream with
all per-core variation in the input data (striped qxT/xr and the 0/1 masks).

Per core:
  - LN1 stats for the whole batch (bn_stats on bf16 x natural) + own queries.
  - LN gain/bias and attention/fc biases are folded host-side:
      ln1(x) @ W = r .* (x @ (g.*W)) + (-mu*r) .* colsum(g.*W) + (b@(g.*W) + b_attn)
    with colsum/beta vectors precomputed on host.
  - q/k kept transposed [d, t]; v natural [t, d] augmented with a ones column
    (at free position 64 for even heads, 63 for odd) so S@V also yields the
    softmax denominator on the partition row matching the head's output rows.
  - causal chunked attention: chunk j attends k-tiles 0..4j+3, last 4 masked.
  - softmax denominators batched per head-pair through a DRAM-roundtrip
    transpose -> [128,16] reciprocal -> partition_broadcast.
  - proj (local, full head dim), residual, LN2 (gain folded into W_fc),
    fc+gelu, fc2, residual.
All matmuls bf16 with f32 PSUM; residual stream and statistics stay f32.
"""
import sys
import math
import contextlib

for _p in ("/opt/trn_rl_repo", "/root/.axon_site/_ro/trn_rl_repo"):
    if _p not in sys.path:
        sys.path.append(_p)

import numpy as np
import ml_dtypes

import concourse.bass as bass
import concourse.bacc as bacc
import concourse.mybir as mybir
import concourse.tile as tile
from concourse.bass_utils import run_bass_kernel_spmd

F32 = mybir.dt.float32
BF16 = mybir.dt.bfloat16
AF = mybir.ActivationFunctionType
OP = mybir.AluOpType
BF = ml_dtypes.bfloat16

B, T, C, H = 4, 2048, 1024, 16
HD = C // H              # 64
EPS = 1e-5
TB = T                   # tokens per batch (2048)
TQ = T // 2              # query tokens per core (1024)
CK = C // 128            # 8 contraction k-tiles over C
NTT = TB // 128          # 16 token tiles per batch
NQT = TQ // 128          # 8 token tiles per core's queries
G = 4                    # head groups
HG = H // G              # 4 heads per group
DG = HG * HD             # 256 cols per group (per q/k/v)
FC = 4 * C               # 4096
NGT = FC // 128          # 32 fc tiles
NCH = 4                  # 256-query chunks per core


def stripe_idx(par):
    return np.concatenate(
        [np.arange(256 * (2 * j + par), 256 * (2 * j + par) + 256)
         for j in range(NCH)])


def build_nc():
    nc = bacc.Bacc("TRN2", target_bir_lowering=False, debug=False, num_devices=8)

    dt_in = {
        # bf16 matmul operands
        "xT": ([C, TB], BF16), "qxT": ([C, TQ], BF16),
        "wqkv": ([C, 3 * C], BF16),
        "wproj": ([C, C], BF16), "wfc": ([C, FC], BF16),
        "wfc2": ([FC, C], BF16), "masks": ([4, 128, 256], BF16),
        # stats inputs (bf16) + residual (f32, b_proj folded)
        "xnb": ([TB, C], BF16), "xrb": ([TQ, C], BF16), "xr": ([TQ, C], F32),
        # host-precomputed LN1/attention fold vectors
        "s_col": ([2 * C, 1], F32), "beta_col": ([2 * C, 1], F32),
        "sv_row": ([1, C], F32), "bv_row": ([1, C], F32),
        # host-folded MLP biases
        "bfc_col": ([FC, 1], F32), "bfc2_bc": ([128, C], F32),
    }
    d = {k: nc.dram_tensor(k, sh, dt, kind="ExternalInput").ap()
         for k, (sh, dt) in dt_in.items()}
    out = nc.dram_tensor("out", [TQ, C], F32, kind="ExternalOutput").ap()

    with tile.TileContext(nc) as tc:
        with contextlib.ExitStack() as ctx:
            _build_body(nc, tc, ctx, d, out)
    nc.compile()
    return nc


def _build_body(nc, tc, ctx, d, out):
    pool = lambda name, bufs, **kw: ctx.enter_context(
        tc.tile_pool(name=name, bufs=bufs, **kw))

    cons = pool("cons", 1)
    small = pool("small", 3)
    rowp = pool("rowp", 2)
    stats = pool("stats", 2)
    ps = pool("ps", 3, space="PSUM")
    dram = pool("dram", 2, space="DRAM")

    # ---- constants / host fold vectors ----
    eps_t = cons.tile([128, 1], F32)
    nc.vector.memset(eps_t, EPS)
    ident = cons.tile([128, 128], BF16)
    from concourse.masks import make_identity
    make_identity(nc, ident)

    scol_sb = cons.tile([128, 16], F32)
    nc.sync.dma_start(out=scol_sb,
                      in_=d["s_col"].rearrange("(k p) o -> p (k o)", p=128))
    bcol_sb = cons.tile([128, 16], F32)
    nc.sync.dma_start(out=bcol_sb,
                      in_=d["beta_col"].rearrange("(k p) o -> p (k o)", p=128))
    sv_bc = cons.tile([128, C], F32)
    bv_bc = cons.tile([128, C], F32)
    with tc.tile_pool(name="svp", bufs=1) as svp:
        svr = svp.tile([1, C], F32, name="svr")
        nc.sync.dma_start(out=svr, in_=d["sv_row"])
        bvr = svp.tile([1, C], F32, name="bvr")
        nc.sync.dma_start(out=bvr, in_=d["bv_row"])
        nc.gpsimd.partition_broadcast(sv_bc, svr)
        nc.gpsimd.partition_broadcast(bv_bc, bvr)

    r_col = cons.tile([128, NTT], F32)
    mr_col = cons.tile([128, NTT], F32)
    rq_col = cons.tile([128, NQT], F32)
    mrq_col = cons.tile([128, NQT], F32)

    # ---- phase 0: LN1 stats (bf16 inputs) ----
    def ln_stats(src_ap, n_tiles, rc, mrc):
        for tt in range(n_tiles):
            xt_f = stats.tile([128, C], BF16, name="xt_f")
            nc.sync.dma_start(out=xt_f, in_=src_ap[tt * 128:(tt + 1) * 128, :])
            st = stats.tile([128, 2, 6], F32, name="st")
            resh = xt_f.rearrange("p (n f) -> p n f", f=512)
            for i in range(2):
                nc.vector.bn_stats(out=st[:, i, :], in_=resh[:, i, :])
            mv = stats.tile([128, 2], F32, name="mv")
            nc.vector.bn_aggr(out=mv, in_=st)
            sd = stats.tile([128, 1], F32, name="sd")
            nc.scalar.activation(sd, mv[:, 1:2], AF.Sqrt, bias=eps_t)
            nc.vector.reciprocal(rc[:, tt:tt + 1], sd)
            nc.vector.tensor_tensor(mrc[:, tt:tt + 1], mv[:, 0:1],
                                    rc[:, tt:tt + 1], op=OP.mult)
            nc.scalar.mul(mrc[:, tt:tt + 1], mrc[:, tt:tt + 1], -1.0)

    ypool = pool("ypool", 1)
    yT2 = ypool.tile([128, H // 2, TQ], BF16)

    attn_ctx0 = contextlib.ExitStack()
    abuf = attn_ctx0.enter_context(tc.tile_pool(name="abuf", bufs=1))
    r_bc = abuf.tile([128, TB], F32)
    mr_bc = abuf.tile([128, TB], F32)
    rq_bc = abuf.tile([128, TQ], F32)
    mrq_bc = abuf.tile([128, TQ], F32)
    masksb = abuf.tile([128, 4, 256], BF16)
    nc.sync.dma_start(out=masksb, in_=d["masks"].rearrange("k p q -> p k q"))

    # ---- attention-scope pools ----
    attn_ctx = contextlib.ExitStack()
    apool = lambda name, bufs, **kw: attn_ctx.enter_context(
        tc.tile_pool(name=name, bufs=bufs, **kw))
    wp = apool("wp", 2)
    xtp = apool("xtp", 2)
    qkv = apool("qkv", 1)
    pp = apool("pp", 16)
    qke = apool("qke", 1)
    dpool = apool("dpool", 1)
    rpool = apool("rpool", 1)
    ybp = apool("ybp", 4)
    psy = apool("psy", 4, space="PSUM")

    qT = qkv.tile([128, HG, TQ], BF16, name="qT")
    kT = qkv.tile([128, HG, TB], BF16, name="kT")
    # va layout [p, tt, parity, idx, 128]: head hg -> (hg%2, hg//2); v weight
    # columns are host-permuted to [h0,h2,h1,h3] per group to match.
    # Even heads: d at 0:64, ones at 64 (denom -> psum partition 64).
    # Odd heads:  d at 64:128, ones at 32 (denom -> psum partition 32).
    va = qkv.tile([128, NTT, 2, 2, 128], BF16, name="va")
    nc.vector.memset(qT, 0.0)
    nc.vector.memset(kT, 0.0)
    nc.vector.memset(va, 0.0)
    nc.vector.memset(va[:, :, 0, :, 64:65], 1.0)
    nc.vector.memset(va[:, :, 1, :, 32:33], 1.0)

    def load_wt(g):
        wt = wp.tile([128, CK, 3 * DG], BF16, name="wt")
        for kt in range(CK):
            for j, base in enumerate((0, C, 2 * C)):
                nc.sync.dma_start(
                    out=wt[:, kt, j * DG:(j + 1) * DG],
                    in_=d["wqkv"][kt * 128:(kt + 1) * 128,
                                  base + g * DG: base + (g + 1) * DG])
        return wt

    def qk_evict(psrc, dst, dt_, sl, rbc_sl, mrbc_sl, sc, bc):
        e1 = qke.tile([128, 512], F32, name="e1")
        nc.vector.tensor_tensor(e1, psrc, rbc_sl, op=OP.mult)
        nc.vector.scalar_tensor_tensor(e1, in0=mrbc_sl, scalar=sc, in1=e1,
                                       op0=OP.mult, op1=OP.add)
        nc.vector.tensor_scalar_add(dst[0:64, 2 * dt_, sl], in0=e1[0:64, :],
                                    scalar1=bc[0:64, :])
        nc.vector.tensor_scalar_add(dst[64:128, 2 * dt_ + 1, sl],
                                    in0=e1[64:128, :], scalar1=bc[64:128, :])

    def load_xt(src_name, ch):
        xt = xtp.tile([128, CK, 512], BF16, name="xt")
        nc.sync.dma_start(
            out=xt,
            in_=d[src_name].rearrange("(k p) t -> p k t", p=128)[:, :, ch * 512:(ch + 1) * 512])
        return xt

    wt_cur = load_wt(0)
    xt_pre = load_xt("xT", 0)

    # stats stream AFTER the first weight/x tiles so the tensor engine can
    # start on group 0 immediately
    ln_stats(d["xnb"], NTT, r_col, mr_col)
    ln_stats(d["xrb"], NQT, rq_col, mrq_col)

    def row_bcast(col_tile, n_tiles, dst):
        scr = dram.tile([n_tiles, 128], F32, name="scr")
        nc.gpsimd.dma_start(out=scr.rearrange("t p -> p t"),
                            in_=col_tile[:, 0:n_tiles])
        flat = scr.rearrange("t p -> (t p)").unsqueeze(0)
        nc.gpsimd.dma_start(out=dst, in_=flat.to_broadcast([128, n_tiles * 128]))

    row_bcast(r_col, NTT, r_bc)
    row_bcast(mr_col, NTT, mr_bc)
    row_bcast(rq_col, NQT, rq_bc)
    row_bcast(mrq_col, NQT, mrq_bc)

    for g in range(G):
        wt = wt_cur
        if g < G - 1:
            wt_cur = load_wt(g + 1)

        # -- k/v over the full batch --
        for ch in range(4):
            if g == 0 and ch == 0:
                xt = xt_pre
            else:
                xt = load_xt("xT", ch)
            for dt_ in range(2):                 # k
                psk = ps.tile([128, 512], F32, name="ps")
                for kt in range(CK):
                    nc.tensor.matmul(psk, wt[:, kt, DG + dt_ * 128:DG + (dt_ + 1) * 128],
                                     xt[:, kt, :], start=(kt == 0), stop=(kt == CK - 1))
                ci = 8 + 2 * g + dt_
                qk_evict(psk, kT, dt_, slice(ch * 512, (ch + 1) * 512),
                         r_bc[:, ch * 512:(ch + 1) * 512],
                         mr_bc[:, ch * 512:(ch + 1) * 512],
                         scol_sb[:, ci:ci + 1], bcol_sb[:, ci:ci + 1])
            for tl in range(4):                  # v (natural layout, par-major)
                tt = ch * 4 + tl
                psv = ps.tile([128, 512], F32, name="ps")
                for kt in range(CK):
                    nc.tensor.matmul(psv[:, 0:DG], xt[:, kt, tl * 128:(tl + 1) * 128],
                                     wt[:, kt, 2 * DG:3 * DG],
                                     start=(kt == 0), stop=(kt == CK - 1))
                zt = qke.tile([128, DG], F32, name="zt")
                nc.vector.scalar_tensor_tensor(zt, in0=sv_bc[:, g * DG:(g + 1) * DG],
                                               scalar=mr_col[:, tt:tt + 1],
                                               in1=bv_bc[:, g * DG:(g + 1) * DG],
                                               op0=OP.mult, op1=OP.add)
                for par, off in ((0, 0), (1, HD)):
                    sl = slice(par * 128, par * 128 + 128)
                    nc.vector.scalar_tensor_tensor(
                        va[:, tt, par, :, off:off + HD],
                        in0=psv[:, sl].rearrange("p (b d) -> p b d", b=2),
                        scalar=r_col[:, tt:tt + 1],
                        in1=zt[:, sl].rearrange("p (b d) -> p b d", b=2),
                        op0=OP.mult, op1=OP.add)
        for ch in range(2):                      # q over own (striped) queries
            qxt = xtp.tile([128, CK, 512], BF16, name="qxt")
            nc.sync.dma_start(
                out=qxt,
                in_=d["qxT"].rearrange("(k p) t -> p k t", p=128)[:, :, ch * 512:(ch + 1) * 512])
            for dt_ in range(2):
                psq = ps.tile([128, 512], F32, name="ps")
                for kt in range(CK):
                    nc.tensor.matmul(psq, wt[:, kt, dt_ * 128:(dt_ + 1) * 128],
                                     qxt[:, kt, :], start=(kt == 0), stop=(kt == CK - 1))
                ci = 2 * g + dt_
                qk_evict(psq, qT, dt_, slice(ch * 512, (ch + 1) * 512),
                         rq_bc[:, ch * 512:(ch + 1) * 512],
                         mrq_bc[:, ch * 512:(ch + 1) * 512],
                         scol_sb[:, ci:ci + 1], bcol_sb[:, ci:ci + 1])

        # -- causal chunk-pair attention, two heads at a time --
        # Chunk pair cp covers chunks {2cp, 2cp+1} = psum cols [0:256|256:512].
        # QK per k-tile is one wide matmul: width 512 while both chunks need
        # the k-tile (kt <= 8cp+3), else 256 (upper chunk only).  SV mirrors
        # this, with the kt==8cp+3 matmul split so the lower chunk's psum
        # accumulation group can close (stop=True) independently.
        def qk_chunk(hg, cp, kt):
            wide = kt <= 8 * cp + 3
            width = 512 if wide else 256
            qoff = cp * 512 + (0 if wide else 256)
            pst = ps.tile([128, 512], F32, name="ps")
            nc.tensor.matmul(pst[:, 0:width],
                             kT[:, hg, kt * 128:(kt + 1) * 128],
                             qT[:, hg, qoff:qoff + width],
                             start=True, stop=True)
            P_t = pp.tile([128, 512], BF16, name="P")
            nc.scalar.activation(P_t[:, 0:width], pst[:, 0:width], AF.Exp,
                                 scale=1.0 / math.sqrt(HD))
            if kt >= 8 * cp:
                nc.vector.tensor_mul(P_t[:, 0:256], P_t[:, 0:256],
                                     masksb[:, kt % 4, :])
            return P_t

        def sv_chunk(hg, cp, psy_p, P_list):
            # Wide accumulation with a range-split tail: arithmetically each
            # psum address sees one start then adds, but the ranges differ per
            # instruction, so the sim's group checker must be bypassed.
            vat = lambda kt: va[:, kt, hg % 2, hg // 2, :]
            mm = lambda o, l, r, st, sp: nc.tensor.matmul(
                o, l, r, start=st, stop=sp, skip_group_check=True)
            last = 8 * cp + 3
            for kt in range(last):
                mm(psy_p, vat(kt), P_list[kt][:, 0:512], kt == 0, False)
            mm(psy_p[:, 0:256], vat(last), P_list[last][:, 0:256], False, True)
            mm(psy_p[:, 256:512], vat(last), P_list[last][:, 256:512], False, False)
            for kt in range(last + 1, 8 * cp + 8):
                mm(psy_p[:, 256:512], vat(kt), P_list[kt][:, 0:256],
                   False, kt == 8 * cp + 7)

        for hp in range(2):
            dsb = dpool.tile([65, 1024], BF16, name="dsb")
            yb_tiles = [[None, None], [None, None]]
            for hi in range(2):
                hg = hp * 2 + hi
                rden = 64 if hg % 2 == 0 else 32
                psy_a = psy.tile([128, 512], F32, name="py")
                psy_b = psy.tile([128, 512], F32, name="py")
                P0 = [qk_chunk(hg, 0, kt) for kt in range(8)]
                P1 = [qk_chunk(hg, 1, kt) for kt in range(8)]
                sv_chunk(hg, 0, psy_a, P0)
                P1 += [qk_chunk(hg, 1, kt) for kt in range(8, 16)]
                sv_chunk(hg, 1, psy_b, P1)
                # evict psum to bf16 SBUF immediately: frees the psum banks so
                # the next head's SV never waits on the denominator roundtrip
                for cp, psy_p in ((0, psy_a), (1, psy_b)):
                    yb = ybp.tile([128, 512], BF16, name="yb")
                    nc.scalar.copy(yb, psy_p)
                    yb_tiles[hi][cp] = yb
                    nc.gpsimd.tensor_copy(
                        dsb[rden:rden + 1, cp * 512:(cp + 1) * 512],
                        yb[rden:rden + 1, :])
            # denominator batch: DRAM transpose -> reciprocal -> broadcast
            scr_d = dram.tile([2, 1024], BF16, name="scr_d")
            nc.sync.dma_start(out=scr_d[0:1, :], in_=dsb[32:33, :])
            nc.sync.dma_start(out=scr_d[1:2, :], in_=dsb[64:65, :])
            den_t = small.tile([128, 16], BF16, name="den_t")
            nc.sync.dma_start(
                out=den_t,
                in_=scr_d.rearrange("r q -> (r q)").rearrange("(p c) -> p c", p=128))
            rec_t = small.tile([128, 16], BF16, name="rec_t")
            with nc.allow_low_precision(reason="softmax denom reciprocal in bf16"):
                nc.vector.reciprocal(rec_t, den_t)
            scr2 = dram.tile([2, 1024], BF16, name="scr2")
            nc.sync.dma_start(
                out=scr2.rearrange("r q -> (r q)").rearrange("(p c) -> p c", p=128),
                in_=rec_t)
            rec_bc = rpool.tile([128, 2, 1024], BF16, name="rec_bc")
            rsb_o = rowp.tile([1, 1024], BF16, name="rsb_o")
            nc.sync.dma_start(out=rsb_o, in_=scr2[0:1, :])
            rsb_e = rowp.tile([1, 1024], BF16, name="rsb_e")
            nc.sync.dma_start(out=rsb_e, in_=scr2[1:2, :])
            nc.gpsimd.partition_broadcast(rec_bc[:, 0, :], rsb_o)
            nc.gpsimd.partition_broadcast(rec_bc[:, 1, :], rsb_e)
            for hi in range(2):
                hg = hp * 2 + hi
                h = g * HG + hg
                rb = (hg % 2) * 64
                pidx = 1 - (hg % 2)
                for j in range(NCH):
                    yb = yb_tiles[hi][j // 2]
                    ysl = slice((j % 2) * 256, (j % 2) * 256 + 256)
                    nc.gpsimd.tensor_tensor(
                        yT2[rb:rb + 64, h // 2, j * 256:(j + 1) * 256],
                        yb[rb:rb + 64, ysl],
                        rec_bc[rb:rb + 64, pidx, j * 256:(j + 1) * 256],
                        op=OP.mult)

    attn_ctx.close()
    attn_ctx0.close()

    # ---- proj + residual (b_proj folded into xr host-side) ----
    mlp = pool("mlp", 1)
    mstr = pool("mstr", 2)
    wstream = pool("wstream", 2)
    c2 = pool("c2", 1)
    x2 = mlp.tile([128, NQT, C], F32)
    hT = mlp.tile([128, CK, TQ], BF16)
    bfc2_sb = c2.tile([128, C], F32)
    nc.sync.dma_start(out=bfc2_sb, in_=d["bfc2_bc"])
    bfc_sb = c2.tile([128, NGT], F32)
    nc.sync.dma_start(out=bfc_sb,
                      in_=d["bfc_col"].rearrange("(k p) o -> p (k o)", p=128))

    wpj_ctx = contextlib.ExitStack()
    wpj_pool = wpj_ctx.enter_context(tc.tile_pool(name="wpj", bufs=1))
    wpj = []
    for kt in range(CK):
        w = wpj_pool.tile([128, C], BF16, name=f"wpj{kt}")
        nc.sync.dma_start(out=w, in_=d["wproj"][kt * 128:(kt + 1) * 128, :])
        wpj.append(w)

    mpsum_ctx = contextlib.ExitStack()
    mpsum = mpsum_ctx.enter_context(
        tc.tile_pool(name="mpsum", bufs=2, space="PSUM"))
    for m in range(NQT):
        xr_t = mstr.tile([128, C], F32, name="xr_t")
        nc.sync.dma_start(out=xr_t, in_=d["xr"][m * 128:(m + 1) * 128, :])
        psp = mpsum.tile([128, 1024], F32, name="mps")
        for n in range(2):
            for kt in range(CK):
                nc.tensor.matmul(psp[:, n * 512:(n + 1) * 512],
                                 yT2[:, kt, m * 128:(m + 1) * 128],
                                 wpj[kt][:, n * 512:(n + 1) * 512],
                                 start=(kt == 0), stop=(kt == CK - 1))
        nc.vector.tensor_tensor(x2[:, m, :], psp, xr_t, op=OP.add)
    wpj_ctx.close()

    # ---- LN2 + transpose (gain/bias folded into W_fc / bfc host-side) ----
    for m in range(NQT):
        st = stats.tile([128, 2, 6], F32, name="st")
        resh = x2[:, m, :].rearrange("p (n f) -> p n f", f=512)
        for i in range(2):
            nc.vector.bn_stats(out=st[:, i, :], in_=resh[:, i, :])
        mv = stats.tile([128, 2], F32, name="mv")
        nc.vector.bn_aggr(out=mv, in_=st)
        sd = stats.tile([128, 1], F32, name="sd")
        nc.scalar.activation(sd, mv[:, 1:2], AF.Sqrt, bias=eps_t)
        r2 = stats.tile([128, 1], F32, name="r2")
        nc.vector.reciprocal(r2, sd)
        hmb = mstr.tile([128, C], BF16, name="hmb")
        nc.vector.tensor_scalar(hmb, in0=x2[:, m, :], scalar1=mv[:, 0:1],
                                scalar2=r2, op0=OP.subtract, op1=OP.mult)
        for ck in range(CK):
            pst = ps.tile([128, 512], F32, name="ps")
            pstv = pst.bitcast(BF16)[:, 0:128]
            nc.tensor.transpose(pstv, hmb[:, ck * 128:(ck + 1) * 128], ident)
            nc.scalar.copy(hT[:, ck, m * 128:(m + 1) * 128], pstv)

    # ---- MLP ----
    # fc1 over both token halves at once (free-1024, weights loaded once)
    hidp = pool("hidp", 1)
    hid = hidp.tile([128, NGT, 1024], BF16, name="hid")
    for gtg in range(NGT // 4):
        wfcg = wstream.tile([128, CK, 512], BF16, name="wfcg")
        for kt in range(CK):
            nc.sync.dma_start(out=wfcg[:, kt, :],
                              in_=d["wfc"][kt * 128:(kt + 1) * 128,
                                           gtg * 512:(gtg + 1) * 512])
        for gi in range(4):
            gt = gtg * 4 + gi
            psf = mpsum.tile([128, 1024], F32, name="mps")
            for th in range(2):
                for kt in range(CK):
                    nc.tensor.matmul(psf[:, th * 512:(th + 1) * 512],
                                     wfcg[:, kt, gi * 128:(gi + 1) * 128],
                                     hT[:, kt, th * 512:(th + 1) * 512],
                                     start=(kt == 0), stop=(kt == CK - 1))
            nc.scalar.activation(hid[:, gt, :], psf, AF.Gelu,
                                 bias=bfc_sb[:, gt:gt + 1])
    mpsum_ctx.close()

    psacc = pool("psacc", 4, space="PSUM")
    wf2p = pool("wf2p", 4)
    ostg = pool("ostg", 2)
    for th in range(2):
        for n in range(2):
            accs = [psacc.tile([128, 512], F32, name="acc") for _ in range(4)]
            for gkt in range(NGT):
                wf2 = wf2p.tile([128, 512], BF16, name="wf2")
                nc.sync.dma_start(out=wf2,
                                  in_=d["wfc2"][gkt * 128:(gkt + 1) * 128,
                                                n * 512:(n + 1) * 512])
                for ml_ in range(4):
                    nc.tensor.matmul(accs[ml_],
                                     hid[:, gkt, th * 512 + ml_ * 128:th * 512 + (ml_ + 1) * 128],
                                     wf2, start=(gkt == 0), stop=(gkt == NGT - 1))
            for ml_ in range(4):
                m = th * 4 + ml_
                osb = ostg.tile([128, 512], F32, name="osb")
                nc.vector.tensor_tensor(osb, accs[ml_], x2[:, m, n * 512:(n + 1) * 512],
                                        op=OP.add)
                nc.vector.tensor_tensor(osb, osb, bfc2_sb[:, n * 512:(n + 1) * 512],
                                        op=OP.add)
                nc.sync.dma_start(out=out[m * 128:(m + 1) * 128, n * 512:(n + 1) * 512],
                                  in_=osb)


def make_masks(par):
    """[4, 128, 256] bf16 for the last 4 k-tiles of every chunk.

    For chunk j the last 4 k-tiles are 4j..4j+3; relative to the chunk's
    queries the masks are j-independent: par=0 -> [diag, diag-128, 0, 0],
    par=1 -> [1, 1, diag, diag-128]."""
    kk = np.arange(128)[:, None]
    qq = np.arange(256)[None, :]
    diag0 = (kk <= qq).astype(np.float32)
    diag1 = (kk + 128 <= qq).astype(np.float32)
    ones = np.ones((128, 256), np.float32)
    zero = np.zeros((128, 256), np.float32)
    pats = [diag0, diag1, zero, zero] if par == 0 else [ones, ones, diag0, diag1]
    return np.stack(pats).astype(BF)


def make_in_maps(inputs):
    f32 = lambda a: np.asarray(a, dtype=np.float32)
    x = f32(inputs["x"])
    W_attn, b_attn = f32(inputs["W_attn"]), f32(inputs["b_attn"])
    W_proj, b_proj = f32(inputs["W_proj"]), f32(inputs["b_proj"])
    W_fc, b_fc = f32(inputs["W_fc"]), f32(inputs["b_fc"])
    W_fc2, b_fc2 = f32(inputs["W_fc2"]), f32(inputs["b_fc2"])
    g1, b1 = f32(inputs["ln1_g"]), f32(inputs["ln1_b"])
    g2, b2 = f32(inputs["ln2_g"]), f32(inputs["ln2_b"])

    g1W = g1[:, None] * W_attn                      # [C, 3C]
    s_all = g1W.sum(axis=0)                         # colsum
    beta_all = b1 @ g1W + b_attn
    # permute v head blocks [h0,h1,h2,h3] -> [h0,h2,h1,h3] per group (va layout)
    perm = np.arange(3 * C)
    for g in range(G):
        base = 2 * C + g * DG
        perm[base:base + DG] = np.concatenate(
            [np.arange(base + b * HD, base + (b + 1) * HD) for b in (0, 2, 1, 3)])
    g1W = g1W[:, perm]
    s_all = s_all[perm]
    beta_all = beta_all[perm]
    g2Wfc = g2[:, None] * W_fc                      # [C, 4C]
    bfc_fold = b2 @ g2Wfc + b_fc

    bc = lambda v: np.ascontiguousarray(np.broadcast_to(v, (128, C)))
    shared = {
        "wqkv": g1W.astype(BF),
        "wproj": W_proj.astype(BF), "wfc": g2Wfc.astype(BF),
        "wfc2": W_fc2.astype(BF),
        "s_col": np.ascontiguousarray(s_all[:2 * C, None]),
        "beta_col": np.ascontiguousarray(beta_all[:2 * C, None]),
        "sv_row": np.ascontiguousarray(s_all[None, 2 * C:]),
        "bv_row": np.ascontiguousarray(beta_all[None, 2 * C:]),
        "bfc_col": np.ascontiguousarray(bfc_fold[:, None]),
        "bfc2_bc": bc(b_fc2),
    }
    masks = {par: make_masks(par) for par in range(2)}
    in_maps = []
    for c in range(8):
        b, par = c // 2, c % 2
        xb = x[b]
        idx = stripe_idx(par)
        xs = xb[idx]
        in_maps.append(dict(
            shared,
            xT=np.ascontiguousarray(xb.T).astype(BF),
            qxT=np.ascontiguousarray(xs.T).astype(BF),
            xnb=xb.astype(BF),
            xrb=xs.astype(BF),
            xr=np.ascontiguousarray(xs + b_proj[None, :]),
            masks=masks[par],
        ))
    return in_maps


def assemble_out(results):
    out = np.empty((B, T, C), np.float32)
    for c in range(8):
        b, par = c // 2, c % 2
        out[b, stripe_idx(par)] = results[c]["out"]
    return out


_NC_CACHE = {}


def kernel(**inputs):
    if "nc" not in _NC_CACHE:
        _NC_CACHE["nc"] = build_nc()
    nc = _NC_CACHE["nc"]
    in_maps = make_in_maps(inputs)
    rr = run_bass_kernel_spmd(nc, in_maps, list(range(8)))
    return assemble_out(rr.results)


# revision 48
# speedup vs baseline: 1.2578x; 1.2578x over previous
"""Trainium2 Bass kernel for a dense transformer block (B=4, T=2048, C=1024, H=16).

Sharding: zero-collective. Each of the 8 cores owns (batch b, query parity par):
  core c -> b = c//2, par = c%2; query chunks = 256-token chunks (2j+par), j=0..3.
Parity striping balances causal attention work exactly: every core runs chunks
whose k-tile counts are (4, 8, 12, 16) -- a uniform SPMD instruction stream with
all per-core variation in the input data (striped qxT/xr and the 0/1 masks).

Per core:
  - LN1 stats for the whole batch (bn_stats on bf16 x natural) + own queries.
  - LN gain/bias and attention/fc biases are folded host-side:
      ln1(x) @ W = r .* (x @ (g.*W)) + (-mu*r) .* colsum(g.*W) + (b@(g.*W) + b_attn)
    with colsum/beta vectors precomputed on host.
  - q/k kept transposed [d, t]; v natural [t, d] augmented with a ones column
    (at free position 64 for even heads, 63 for odd) so S@V also yields the
    softmax denominator on the partition row matching the head's output rows.
  - causal chunked attention: chunk j attends k-tiles 0..4j+3, last 4 masked.
  - softmax denominators batched per head-pair through a DRAM-roundtrip
    transpose -> [128,16] reciprocal -> partition_broadcast.
  - proj (local, full head dim), residual, LN2 (gain folded into W_fc),
    fc+gelu, fc2, residual.
All matmuls bf16 with f32 PSUM; residual stream and statistics stay f32.
"""
import sys
import math
import contextlib

for _p in ("/opt/trn_rl_repo", "/root/.axon_site/_ro/trn_rl_repo"):
    if _p not in sys.path:
        sys.path.append(_p)

import numpy as np
import ml_dtypes

import concourse.bass as bass
import concourse.bacc as bacc
import concourse.mybir as mybir
import concourse.tile as tile
from concourse.bass_utils import run_bass_kernel_spmd

F32 = mybir.dt.float32
BF16 = mybir.dt.bfloat16
AF = mybir.ActivationFunctionType
OP = mybir.AluOpType
BF = ml_dtypes.bfloat16

B, T, C, H = 4, 2048, 1024, 16
HD = C // H              # 64
EPS = 1e-5
TB = T                   # tokens per batch (2048)
TQ = T // 2              # query tokens per core (1024)
CK = C // 128            # 8 contraction k-tiles over C
NTT = TB // 128          # 16 token tiles per batch
NQT = TQ // 128          # 8 token tiles per core's queries
G = 4                    # head groups
HG = H // G              # 4 heads per group
DG = HG * HD             # 256 cols per group (per q/k/v)
FC = 4 * C               # 4096
NGT = FC // 128          # 32 fc tiles
NCH = 4                  # 256-query chunks per core


def stripe_idx(par):
    return np.concatenate(
        [np.arange(256 * (2 * j + par), 256 * (2 * j + par) + 256)
         for j in range(NCH)])


def build_nc():
    nc = bacc.Bacc("TRN2", target_bir_lowering=False, debug=False, num_devices=8)

    dt_in = {
        # bf16 matmul operands
        "xT": ([C, TB], BF16), "qxT": ([C, TQ], BF16),
        "wqkv": ([C, 3 * C], BF16),
        "wproj": ([C, C], BF16), "wfc": ([C, FC], BF16),
        "wfc2": ([FC, C], BF16), "masks": ([4, 128, 256], BF16),
        # stats inputs (bf16) + residual (f32, b_proj folded)
        "xnb": ([TB, C], BF16), "xrb": ([TQ, C], BF16), "xr": ([TQ, C], F32),
        # host-precomputed LN1/attention fold vectors
        "s_col": ([2 * C, 1], F32), "beta_col": ([2 * C, 1], F32),
        "sv_row": ([1, C], F32), "bv_row": ([1, C], F32),
        # host-folded MLP biases
        "bfc_col": ([FC, 1], F32), "bfc2_bc": ([128, C], F32),
    }
    d = {k: nc.dram_tensor(k, sh, dt, kind="ExternalInput").ap()
         for k, (sh, dt) in dt_in.items()}
    out = nc.dram_tensor("out", [TQ, C], F32, kind="ExternalOutput").ap()

    with tile.TileContext(nc) as tc:
        with contextlib.ExitStack() as ctx:
            _build_body(nc, tc, ctx, d, out)
    nc.compile()
    return nc


def _build_body(nc, tc, ctx, d, out):
    pool = lambda name, bufs, **kw: ctx.enter_context(
        tc.tile_pool(name=name, bufs=bufs, **kw))

    cons = pool("cons", 1)
    small = pool("small", 3)
    rowp = pool("rowp", 2)
    stats = pool("stats", 2)
    ps = pool("ps", 3, space="PSUM")
    dram = pool("dram", 2, space="DRAM")

    # ---- constants / host fold vectors ----
    eps_t = cons.tile([128, 1], F32)
    nc.vector.memset(eps_t, EPS)
    ident = cons.tile([128, 128], BF16)
    from concourse.masks import make_identity
    make_identity(nc, ident)

    scol_sb = cons.tile([128, 16], F32)
    nc.sync.dma_start(out=scol_sb,
                      in_=d["s_col"].rearrange("(k p) o -> p (k o)", p=128))
    bcol_sb = cons.tile([128, 16], F32)
    nc.sync.dma_start(out=bcol_sb,
                      in_=d["beta_col"].rearrange("(k p) o -> p (k o)", p=128))
    sv_bc = cons.tile([128, C], F32)
    bv_bc = cons.tile([128, C], F32)
    with tc.tile_pool(name="svp", bufs=1) as svp:
        svr = svp.tile([1, C], F32, name="svr")
        nc.sync.dma_start(out=svr, in_=d["sv_row"])
        bvr = svp.tile([1, C], F32, name="bvr")
        nc.sync.dma_start(out=bvr, in_=d["bv_row"])
        nc.gpsimd.partition_broadcast(sv_bc, svr)
        nc.gpsimd.partition_broadcast(bv_bc, bvr)

    r_col = cons.tile([128, NTT], F32)
    mr_col = cons.tile([128, NTT], F32)
    rq_col = cons.tile([128, NQT], F32)
    mrq_col = cons.tile([128, NQT], F32)

    # ---- phase 0: LN1 stats (bf16 inputs) ----
    def ln_stats(src_ap, n_tiles, rc, mrc):
        for tt in range(n_tiles):
            xt_f = stats.tile([128, C], BF16, name="xt_f")
            nc.sync.dma_start(out=xt_f, in_=src_ap[tt * 128:(tt + 1) * 128, :])
            st = stats.tile([128, 2, 6], F32, name="st")
            resh = xt_f.rearrange("p (n f) -> p n f", f=512)
            for i in range(2):
                nc.vector.bn_stats(out=st[:, i, :], in_=resh[:, i, :])
            mv = stats.tile([128, 2], F32, name="mv")
            nc.vector.bn_aggr(out=mv, in_=st)
            sd = stats.tile([128, 1], F32, name="sd")
            nc.scalar.activation(sd, mv[:, 1:2], AF.Sqrt, bias=eps_t)
            nc.vector.reciprocal(rc[:, tt:tt + 1], sd)
            nc.vector.tensor_tensor(mrc[:, tt:tt + 1], mv[:, 0:1],
                                    rc[:, tt:tt + 1], op=OP.mult)
            nc.scalar.mul(mrc[:, tt:tt + 1], mrc[:, tt:tt + 1], -1.0)

    ypool = pool("ypool", 1)
    yT2 = ypool.tile([128, H // 2, TQ], BF16)

    attn_ctx0 = contextlib.ExitStack()
    abuf = attn_ctx0.enter_context(tc.tile_pool(name="abuf", bufs=1))
    r_bc = abuf.tile([128, TB], F32)
    mr_bc = abuf.tile([128, TB], F32)
    rq_bc = abuf.tile([128, TQ], F32)
    mrq_bc = abuf.tile([128, TQ], F32)
    masksb = abuf.tile([128, 4, 256], BF16)
    nc.sync.dma_start(out=masksb, in_=d["masks"].rearrange("k p q -> p k q"))

    # ---- attention-scope pools ----
    attn_ctx = contextlib.ExitStack()
    apool = lambda name, bufs, **kw: attn_ctx.enter_context(
        tc.tile_pool(name=name, bufs=bufs, **kw))
    wp = apool("wp", 2)
    xtp = apool("xtp", 2)
    qkv = apool("qkv", 1)
    pp = apool("pp", 16)
    qke = apool("qke", 1)
    dpool = apool("dpool", 1)
    rpool = apool("rpool", 2)
    ybp = apool("ybp", 8)
    psy = apool("psy", 4, space="PSUM")

    qT = qkv.tile([128, HG, TQ], BF16, name="qT")
    kT = qkv.tile([128, HG, TB], BF16, name="kT")
    # va layout [p, tt, parity, idx, 128]: head hg -> (hg%2, hg//2); v weight
    # columns are host-permuted to [h0,h2,h1,h3] per group to match.
    # Even heads: d at 0:64, ones at 64 (denom -> psum partition 64).
    # Odd heads:  d at 64:128, ones at 32 (denom -> psum partition 32).
    va = qkv.tile([128, NTT, 2, 2, 128], BF16, name="va")
    nc.vector.memset(qT, 0.0)
    nc.vector.memset(kT, 0.0)
    nc.vector.memset(va, 0.0)
    nc.vector.memset(va[:, :, 0, :, 64:65], 1.0)
    nc.vector.memset(va[:, :, 1, :, 32:33], 1.0)

    def load_wt(g):
        wt = wp.tile([128, CK, 3 * DG], BF16, name="wt")
        for kt in range(CK):
            for j, base in enumerate((0, C, 2 * C)):
                nc.sync.dma_start(
                    out=wt[:, kt, j * DG:(j + 1) * DG],
                    in_=d["wqkv"][kt * 128:(kt + 1) * 128,
                                  base + g * DG: base + (g + 1) * DG])
        return wt

    def qk_evict(psrc, dst, dt_, sl, rbc_sl, mrbc_sl, sc, bc):
        e1 = qke.tile([128, 512], F32, name="e1")
        nc.vector.tensor_tensor(e1, psrc, rbc_sl, op=OP.mult)
        nc.vector.scalar_tensor_tensor(e1, in0=mrbc_sl, scalar=sc, in1=e1,
                                       op0=OP.mult, op1=OP.add)
        nc.vector.tensor_scalar_add(dst[0:64, 2 * dt_, sl], in0=e1[0:64, :],
                                    scalar1=bc[0:64, :])
        nc.vector.tensor_scalar_add(dst[64:128, 2 * dt_ + 1, sl],
                                    in0=e1[64:128, :], scalar1=bc[64:128, :])

    def load_xt(src_name, ch):
        xt = xtp.tile([128, CK, 512], BF16, name="xt")
        nc.sync.dma_start(
            out=xt,
            in_=d[src_name].rearrange("(k p) t -> p k t", p=128)[:, :, ch * 512:(ch + 1) * 512])
        return xt

    def emit_yst(yb_tiles, rec_bc, g, hp):
        for hi in range(2):
            hg = hp * 2 + hi
            h = g * HG + hg
            rb = (hg % 2) * 64
            pidx = 1 - (hg % 2)
            for j in range(NCH):
                yb = yb_tiles[hi][j // 2]
                ysl = slice((j % 2) * 256, (j % 2) * 256 + 256)
                nc.vector.tensor_tensor(
                    yT2[rb:rb + 64, h // 2, j * 256:(j + 1) * 256],
                    yb[rb:rb + 64, ysl],
                    rec_bc[rb:rb + 64, pidx, j * 256:(j + 1) * 256],
                    op=OP.mult)

    pending_yst = None
    wt_cur = load_wt(0)
    xt_pre = load_xt("xT", 0)

    # stats stream AFTER the first weight/x tiles so the tensor engine can
    # start on group 0 immediately
    ln_stats(d["xnb"], NTT, r_col, mr_col)
    ln_stats(d["xrb"], NQT, rq_col, mrq_col)

    def row_bcast(col_tile, n_tiles, dst):
        scr = dram.tile([n_tiles, 128], F32, name="scr")
        nc.gpsimd.dma_start(out=scr.rearrange("t p -> p t"),
                            in_=col_tile[:, 0:n_tiles])
        flat = scr.rearrange("t p -> (t p)").unsqueeze(0)
        nc.sync.dma_start(out=dst, in_=flat.to_broadcast([128, n_tiles * 128]))

    row_bcast(r_col, NTT, r_bc)
    row_bcast(mr_col, NTT, mr_bc)
    row_bcast(rq_col, NQT, rq_bc)
    row_bcast(mrq_col, NQT, mrq_bc)

    for g in range(G):
        wt = wt_cur
        if g < G - 1:
            wt_cur = load_wt(g + 1)

        # -- k/v over the full batch --
        for ch in range(4):
            if g == 0 and ch == 0:
                xt = xt_pre
            else:
                xt = load_xt("xT", ch)
            for dt_ in range(2):                 # k
                psk = ps.tile([128, 512], F32, name="ps")
                for kt in range(CK):
                    nc.tensor.matmul(psk, wt[:, kt, DG + dt_ * 128:DG + (dt_ + 1) * 128],
                                     xt[:, kt, :], start=(kt == 0), stop=(kt == CK - 1))
                ci = 8 + 2 * g + dt_
                qk_evict(psk, kT, dt_, slice(ch * 512, (ch + 1) * 512),
                         r_bc[:, ch * 512:(ch + 1) * 512],
                         mr_bc[:, ch * 512:(ch + 1) * 512],
                         scol_sb[:, ci:ci + 1], bcol_sb[:, ci:ci + 1])
            for tl in range(4):                  # v (natural layout, par-major)
                tt = ch * 4 + tl
                psv = ps.tile([128, 512], F32, name="ps")
                for kt in range(CK):
                    nc.tensor.matmul(psv[:, 0:DG], xt[:, kt, tl * 128:(tl + 1) * 128],
                                     wt[:, kt, 2 * DG:3 * DG],
                                     start=(kt == 0), stop=(kt == CK - 1))
                zt = qke.tile([128, DG], F32, name="zt")
                nc.vector.scalar_tensor_tensor(zt, in0=sv_bc[:, g * DG:(g + 1) * DG],
                                               scalar=mr_col[:, tt:tt + 1],
                                               in1=bv_bc[:, g * DG:(g + 1) * DG],
                                               op0=OP.mult, op1=OP.add)
                for par, off in ((0, 0), (1, HD)):
                    sl = slice(par * 128, par * 128 + 128)
                    nc.vector.scalar_tensor_tensor(
                        va[:, tt, par, :, off:off + HD],
                        in0=psv[:, sl].rearrange("p (b d) -> p b d", b=2),
                        scalar=r_col[:, tt:tt + 1],
                        in1=zt[:, sl].rearrange("p (b d) -> p b d", b=2),
                        op0=OP.mult, op1=OP.add)
        for ch in range(2):                      # q over own (striped) queries
            qxt = load_xt("qxT", ch)
            for dt_ in range(2):
                psq = ps.tile([128, 512], F32, name="ps")
                for kt in range(CK):
                    nc.tensor.matmul(psq, wt[:, kt, dt_ * 128:(dt_ + 1) * 128],
                                     qxt[:, kt, :], start=(kt == 0), stop=(kt == CK - 1))
                ci = 2 * g + dt_
                qk_evict(psq, qT, dt_, slice(ch * 512, (ch + 1) * 512),
                         rq_bc[:, ch * 512:(ch + 1) * 512],
                         mrq_bc[:, ch * 512:(ch + 1) * 512],
                         scol_sb[:, ci:ci + 1], bcol_sb[:, ci:ci + 1])

        # -- causal chunk-pair attention, two heads at a time --
        # Chunk pair cp covers chunks {2cp, 2cp+1} = psum cols [0:256|256:512].
        # QK per k-tile is one wide matmul: width 512 while both chunks need
        # the k-tile (kt <= 8cp+3), else 256 (upper chunk only).  SV mirrors
        # this, with the kt==8cp+3 matmul split so the lower chunk's psum
        # accumulation group can close (stop=True) independently.
        def qk_chunk(hg, cp, kt):
            wide = kt <= 8 * cp + 3
            width = 512 if wide else 256
            qoff = cp * 512 + (0 if wide else 256)
            pst = ps.tile([128, 512], F32, name="ps")
            nc.tensor.matmul(pst[:, 0:width],
                             kT[:, hg, kt * 128:(kt + 1) * 128],
                             qT[:, hg, qoff:qoff + width],
                             start=True, stop=True)
            P_t = pp.tile([128, 512], BF16, name="P")
            nc.scalar.activation(P_t[:, 0:width], pst[:, 0:width], AF.Exp,
                                 scale=1.0 / math.sqrt(HD))
            if kt >= 8 * cp:
                nc.vector.tensor_mul(P_t[:, 0:256], P_t[:, 0:256],
                                     masksb[:, kt % 4, :])
            return P_t

        def sv_chunk(hg, cp, psy_p, P_list):
            # Wide accumulation with a range-split tail: arithmetically each
            # psum address sees one start then adds, but the ranges differ per
            # instruction, so the sim's group checker must be bypassed.
            vat = lambda kt: va[:, kt, hg % 2, hg // 2, :]
            mm = lambda o, l, r, st, sp: nc.tensor.matmul(
                o, l, r, start=st, stop=sp, skip_group_check=True)
            last = 8 * cp + 3
            for kt in range(last):
                mm(psy_p, vat(kt), P_list[kt][:, 0:512], kt == 0, False)
            mm(psy_p[:, 0:256], vat(last), P_list[last][:, 0:256], False, True)
            mm(psy_p[:, 256:512], vat(last), P_list[last][:, 256:512], False, False)
            for kt in range(last + 1, 8 * cp + 8):
                mm(psy_p[:, 256:512], vat(kt), P_list[kt][:, 0:256],
                   False, kt == 8 * cp + 7)

        for hp in range(2):
            dsb = dpool.tile([65, 1024], BF16, name="dsb")
            yb_tiles = [[None, None], [None, None]]
            for hi in range(2):
                hg = hp * 2 + hi
                rden = 64 if hg % 2 == 0 else 32
                psy_a = psy.tile([128, 512], F32, name="py")
                psy_b = psy.tile([128, 512], F32, name="py")
                P0 = [qk_chunk(hg, 0, kt) for kt in range(8)]
                P1 = [qk_chunk(hg, 1, kt) for kt in range(8)]
                sv_chunk(hg, 0, psy_a, P0)
                P1 += [qk_chunk(hg, 1, kt) for kt in range(8, 16)]
                sv_chunk(hg, 1, psy_b, P1)
                # evict psum to bf16 SBUF immediately: frees the psum banks so
                # the next head's SV never waits on the denominator roundtrip
                for cp, psy_p in ((0, psy_a), (1, psy_b)):
                    nc.vector.tensor_copy(
                        dsb[rden:rden + 1, cp * 512:(cp + 1) * 512],
                        psy_p[rden:rden + 1, :])
                    yb = ybp.tile([128, 512], BF16, name="yb")
                    nc.scalar.copy(yb, psy_p)
                    yb_tiles[hi][cp] = yb
            # denominator batch: DRAM transpose -> reciprocal -> broadcast
            scr_d = dram.tile([2, 1024], BF16, name="scr_d")
            nc.sync.dma_start(out=scr_d[0:1, :], in_=dsb[32:33, :])
            nc.sync.dma_start(out=scr_d[1:2, :], in_=dsb[64:65, :])
            den_t = small.tile([128, 16], BF16, name="den_t")
            nc.sync.dma_start(
                out=den_t,
                in_=scr_d.rearrange("r q -> (r q)").rearrange("(p c) -> p c", p=128))
            rec_t = small.tile([128, 16], BF16, name="rec_t")
            with nc.allow_low_precision(reason="softmax denom reciprocal in bf16"):
                nc.vector.reciprocal(rec_t, den_t)
            scr2 = dram.tile([2, 1024], BF16, name="scr2")
            nc.sync.dma_start(
                out=scr2.rearrange("r q -> (r q)").rearrange("(p c) -> p c", p=128),
                in_=rec_t)
            rec_bc = rpool.tile([128, 2, 1024], BF16, name="rec_bc")
            rsb_o = rowp.tile([1, 1024], BF16, name="rsb_o")
            nc.sync.dma_start(out=rsb_o, in_=scr2[0:1, :])
            rsb_e = rowp.tile([1, 1024], BF16, name="rsb_e")
            nc.sync.dma_start(out=rsb_e, in_=scr2[1:2, :])
            nc.gpsimd.partition_broadcast(rec_bc[:, 0, :], rsb_o)
            nc.gpsimd.partition_broadcast(rec_bc[:, 1, :], rsb_e)
            # defer this pair's normalization by one pair so the vector queue
            # never stalls on the reciprocal roundtrip
            if pending_yst is not None:
                emit_yst(*pending_yst)
            pending_yst = (yb_tiles, rec_bc, g, hp)

    emit_yst(*pending_yst)
    attn_ctx.close()
    attn_ctx0.close()

    # ---- proj + residual (b_proj folded into xr host-side) ----
    mlp = pool("mlp", 1)
    mstr = pool("mstr", 2)
    wstream = pool("wstream", 2)
    c2 = pool("c2", 1)
    x2 = mlp.tile([128, NQT, C], F32)
    hT = mlp.tile([128, CK, TQ], BF16)
    bfc2_sb = c2.tile([128, C], F32)
    nc.sync.dma_start(out=bfc2_sb, in_=d["bfc2_bc"])
    bfc_sb = c2.tile([128, NGT], F32)
    nc.sync.dma_start(out=bfc_sb,
                      in_=d["bfc_col"].rearrange("(k p) o -> p (k o)", p=128))

    wpj_ctx = contextlib.ExitStack()
    wpj_pool = wpj_ctx.enter_context(tc.tile_pool(name="wpj", bufs=1))
    wpj = []
    for kt in range(CK):
        w = wpj_pool.tile([128, C], BF16, name=f"wpj{kt}")
        nc.sync.dma_start(out=w, in_=d["wproj"][kt * 128:(kt + 1) * 128, :])
        wpj.append(w)

    mpsum_ctx = contextlib.ExitStack()
    mpsum = mpsum_ctx.enter_context(
        tc.tile_pool(name="mpsum", bufs=2, space="PSUM"))
    for m in range(NQT):
        xr_t = mstr.tile([128, C], F32, name="xr_t")
        nc.sync.dma_start(out=xr_t, in_=d["xr"][m * 128:(m + 1) * 128, :])
        psp = mpsum.tile([128, 1024], F32, name="mps")
        for n in range(2):
            for kt in range(CK):
                nc.tensor.matmul(psp[:, n * 512:(n + 1) * 512],
                                 yT2[:, kt, m * 128:(m + 1) * 128],
                                 wpj[kt][:, n * 512:(n + 1) * 512],
                                 start=(kt == 0), stop=(kt == CK - 1))
        nc.vector.tensor_tensor(x2[:, m, :], psp, xr_t, op=OP.add)
    wpj_ctx.close()

    # ---- LN2 + transpose (gain/bias folded into W_fc / bfc host-side) ----
    for m in range(NQT):
        st = stats.tile([128, 2, 6], F32, name="st")
        resh = x2[:, m, :].rearrange("p (n f) -> p n f", f=512)
        for i in range(2):
            nc.vector.bn_stats(out=st[:, i, :], in_=resh[:, i, :])
        mv = stats.tile([128, 2], F32, name="mv")
        nc.vector.bn_aggr(out=mv, in_=st)
        sd = stats.tile([128, 1], F32, name="sd")
        nc.scalar.activation(sd, mv[:, 1:2], AF.Sqrt, bias=eps_t)
        r2 = stats.tile([128, 1], F32, name="r2")
        nc.vector.reciprocal(r2, sd)
        hmb = mstr.tile([128, C], BF16, name="hmb")
        nc.vector.tensor_scalar(hmb, in0=x2[:, m, :], scalar1=mv[:, 0:1],
                                scalar2=r2, op0=OP.subtract, op1=OP.mult)
        for ck in range(CK):
            pst = ps.tile([128, 512], F32, name="ps")
            pstv = pst.bitcast(BF16)[:, 0:128]
            nc.tensor.transpose(pstv, hmb[:, ck * 128:(ck + 1) * 128], ident)
            nc.scalar.copy(hT[:, ck, m * 128:(m + 1) * 128], pstv)

    # ---- MLP ----
    # fc1 over both token halves at once (free-1024, weights loaded once)
    hidp = pool("hidp", 1)
    hid = hidp.tile([128, NGT, 1024], BF16, name="hid")
    for gtg in range(NGT // 4):
        wfcg = wstream.tile([128, CK, 512], BF16, name="wfcg")
        for kt in range(CK):
            nc.sync.dma_start(out=wfcg[:, kt, :],
                              in_=d["wfc"][kt * 128:(kt + 1) * 128,
                                           gtg * 512:(gtg + 1) * 512])
        for gi in range(4):
            gt = gtg * 4 + gi
            psf = mpsum.tile([128, 1024], F32, name="mps")
            for th in range(2):
                for kt in range(CK):
                    nc.tensor.matmul(psf[:, th * 512:(th + 1) * 512],
                                     wfcg[:, kt, gi * 128:(gi + 1) * 128],
                                     hT[:, kt, th * 512:(th + 1) * 512],
                                     start=(kt == 0), stop=(kt == CK - 1))
            nc.scalar.activation(hid[:, gt, :], psf, AF.Gelu,
                                 bias=bfc_sb[:, gt:gt + 1])
    mpsum_ctx.close()

    psacc = pool("psacc", 4, space="PSUM")
    wf2p = pool("wf2p", 4)
    ostg = pool("ostg", 2)
    for th in range(2):
        for n in range(2):
            accs = [psacc.tile([128, 512], F32, name="acc") for _ in range(4)]
            for gkt in range(NGT):
                wf2 = wf2p.tile([128, 512], BF16, name="wf2")
                nc.sync.dma_start(out=wf2,
                                  in_=d["wfc2"][gkt * 128:(gkt + 1) * 128,
                                                n * 512:(n + 1) * 512])
                for ml_ in range(4):
                    nc.tensor.matmul(accs[ml_],
                                     hid[:, gkt, th * 512 + ml_ * 128:th * 512 + (ml_ + 1) * 128],
                                     wf2, start=(gkt == 0), stop=(gkt == NGT - 1))
            for ml_ in range(4):
                m = th * 4 + ml_
                osb = ostg.tile([128, 512], F32, name="osb")
                nc.vector.tensor_tensor(osb, accs[ml_], x2[:, m, n * 512:(n + 1) * 512],
                                        op=OP.add)
                nc.vector.tensor_tensor(osb, osb, bfc2_sb[:, n * 512:(n + 1) * 512],
                                        op=OP.add)
                nc.sync.dma_start(out=out[m * 128:(m + 1) * 128, n * 512:(n + 1) * 512],
                                  in_=osb)


def make_masks(par):
    """[4, 128, 256] bf16 for the last 4 k-tiles of every chunk.

    For chunk j the last 4 k-tiles are 4j..4j+3; relative to the chunk's
    queries the masks are j-independent: par=0 -> [diag, diag-128, 0, 0],
    par=1 -> [1, 1, diag, diag-128]."""
    kk = np.arange(128)[:, None]
    qq = np.arange(256)[None, :]
    diag0 = (kk <= qq).astype(np.float32)
    diag1 = (kk + 128 <= qq).astype(np.float32)
    ones = np.ones((128, 256), np.float32)
    zero = np.zeros((128, 256), np.float32)
    pats = [diag0, diag1, zero, zero] if par == 0 else [ones, ones, diag0, diag1]
    return np.stack(pats).astype(BF)


def make_in_maps(inputs):
    f32 = lambda a: np.asarray(a, dtype=np.float32)
    x = f32(inputs["x"])
    W_attn, b_attn = f32(inputs["W_attn"]), f32(inputs["b_attn"])
    W_proj, b_proj = f32(inputs["W_proj"]), f32(inputs["b_proj"])
    W_fc, b_fc = f32(inputs["W_fc"]), f32(inputs["b_fc"])
    W_fc2, b_fc2 = f32(inputs["W_fc2"]), f32(inputs["b_fc2"])
    g1, b1 = f32(inputs["ln1_g"]), f32(inputs["ln1_b"])
    g2, b2 = f32(inputs["ln2_g"]), f32(inputs["ln2_b"])

    g1W = g1[:, None] * W_attn                      # [C, 3C]
    s_all = g1W.sum(axis=0)                         # colsum
    beta_all = b1 @ g1W + b_attn
    # permute v head blocks [h0,h1,h2,h3] -> [h0,h2,h1,h3] per group (va layout)
    perm = np.arange(3 * C)
    for g in range(G):
        base = 2 * C + g * DG
        perm[base:base + DG] = np.concatenate(
            [np.arange(base + b * HD, base + (b + 1) * HD) for b in (0, 2, 1, 3)])
    g1W = g1W[:, perm]
    s_all = s_all[perm]
    beta_all = beta_all[perm]
    g2Wfc = g2[:, None] * W_fc                      # [C, 4C]
    bfc_fold = b2 @ g2Wfc + b_fc

    bc = lambda v: np.ascontiguousarray(np.broadcast_to(v, (128, C)))
    shared = {
        "wqkv": g1W.astype(BF),
        "wproj": W_proj.astype(BF), "wfc": g2Wfc.astype(BF),
        "wfc2": W_fc2.astype(BF),
        "s_col": np.ascontiguousarray(s_all[:2 * C, None]),
        "beta_col": np.ascontiguousarray(beta_all[:2 * C, None]),
        "sv_row": np.ascontiguousarray(s_all[None, 2 * C:]),
        "bv_row": np.ascontiguousarray(beta_all[None, 2 * C:]),
        "bfc_col": np.ascontiguousarray(bfc_fold[:, None]),
        "bfc2_bc": bc(b_fc2),
    }
    masks = {par: make_masks(par) for par in range(2)}
    in_maps = []
    for c in range(8):
        b, par = c // 2, c % 2
        xb = x[b]
        idx = stripe_idx(par)
        xs = xb[idx]
        in_maps.append(dict(
            shared,
            xT=np.ascontiguousarray(xb.T).astype(BF),
            qxT=np.ascontiguousarray(xs.T).astype(BF),
            xnb=xb.astype(BF),
            xrb=xs.astype(BF),
            xr=np.ascontiguousarray(xs + b_proj[None, :]),
            masks=masks[par],
        ))
    return in_maps


def assemble_out(results):
    out = np.empty((B, T, C), np.float32)
    for c in range(8):
        b, par = c // 2, c % 2
        out[b, stripe_idx(par)] = results[c]["out"]
    return out


_NC_CACHE = {}


def kernel(**inputs):
    if "nc" not in _NC_CACHE:
        _NC_CACHE["nc"] = build_nc()
    nc = _NC_CACHE["nc"]
    in_maps = make_in_maps(inputs)
    rr = run_bass_kernel_spmd(nc, in_maps, list(range(8)))
    return assemble_out(rr.results)
